# revision 26
# baseline (speedup 1.0000x reference)
"""Trainium2 Bass kernel for multi-head self-attention (B=2, S=2048, 16 heads).

Sharding: 8 cores = (2 batches) x (4 query-blocks of 512). Each core computes
Q for its 512 queries against all 2048 keys for all 16 heads, plus the output
projection rows for its queries. Outputs are disjoint shards -> no collectives.

Per-core layout notes:
  - x is fed pre-transposed (xT [1024, 2048]) so projections need no on-chip
    transpose of the activations.
  - W_Q/W_K/W_V columns (and b_*) are permuted on host to head-major order
    (reference uses head-minor "(h n)" packing).
  - softmax runs without row-max subtraction (scores are bounded; masked
    entries are exactly -1e26 -> exp -> 0). Pad-masked query rows use a
    per-partition exp scale of 0 so every entry becomes exp(0)=1 and the row
    normalizes to exactly 1/2048, matching the reference bit pattern.
"""

import os
import sys

sys.path.insert(0, "/opt/trn_rl_repo")

import numpy as np
import ml_dtypes

import concourse.bass as bass
import concourse.mybir as mybir
import concourse.tile as tile
from concourse.bass_utils import run_bass_kernel_spmd

B, S, INP, HID, NH, OUT = 2, 2048, 1024, 64, 16, 1024
NEG = np.float32(-1e26)
SQ = 512          # queries per core
NCORES = 8
P = 128

F32 = mybir.dt.float32
F32R = mybir.dt.float32r
BF16 = mybir.dt.bfloat16

# head-major permutation: permuted col h*64+d  <- original col d*16+h
PERM = np.array([d * NH + h for h in range(NH) for d in range(HID)], dtype=np.int64)


def _split_multi_waits(nc):
    """Legalize sync waits for this walrus build: every instruction except
    Drain may carry at most 1 sem wait (EventSemaphore: 2). Excess waits are
    hoisted onto same-engine NOPs inserted immediately before the instruction
    (engine blocks on each in turn -> identical semantics)."""
    import bass_rust

    cnt = [0]
    for f in nc.m.functions:
        for blk in f.blocks:
            il = blk.instructions
            i = 0
            while i < len(il):
                ins = il[i]
                tn = type(ins).__name__
                si = ins.sync_info
                cap = 2 if tn == "InstEventSemaphore" else 1
                if si is None or len(si.on_wait) <= cap:
                    i += 1
                    continue
                waits = list(si.on_wait)
                extra, kept = waits[:-cap], waits[-cap:]
                ins.sync_info = bass_rust.SyncInfo(
                    on_wait=kept, on_update=si.on_update)
                for w in extra:
                    cnt[0] += 1
                    nop = mybir.InstNoOp(
                        name=f"I-waitsplit-{cnt[0]}",
                        engine=ins.engine,
                        ins=[], outs=[],
                        sync_info=bass_rust.SyncInfo(on_wait=[w], on_update=[]))
                    il.insert(i, nop)
                    i += 1
                i += 1
    return cnt[0]


def build_nc(legalize=True):
    nc = bass.Bass()

    xT = nc.declare_dram_parameter("xT", [INP, S], BF16, isOutput=False)
    xq = nc.declare_dram_parameter("xq", [INP, SQ], BF16, isOutput=False)
    wq = nc.declare_dram_parameter("wq", [INP, NH * HID], BF16, isOutput=False)
    wk = nc.declare_dram_parameter("wk", [INP, NH * HID], BF16, isOutput=False)
    wv = nc.declare_dram_parameter("wv", [INP, NH * HID], BF16, isOutput=False)
    wo = nc.declare_dram_parameter("wo", [NH * HID, OUT], BF16, isOutput=False)
    bq = nc.declare_dram_parameter("bq", [P, INP // P], F32, isOutput=False)
    bk = nc.declare_dram_parameter("bk", [P, INP // P], F32, isOutput=False)
    vbias = nc.declare_dram_parameter("vbias", [P, NH * HID], F32, isOutput=False)
    obias = nc.declare_dram_parameter("obias", [P, OUT], F32, isOutput=False)
    maskM = nc.declare_dram_parameter("maskM", [SQ, S], BF16, isOutput=False)
    qscale = nc.declare_dram_parameter("qscale", [P, SQ // P], F32, isOutput=False)
    ident32 = nc.declare_dram_parameter("ident32", [P, P], F32, isOutput=False)
    identbf = nc.declare_dram_parameter("identbf", [P, P], BF16, isOutput=False)

    attn_o = nc.declare_dram_parameter("attn_o", [NH, SQ, S], F32, isOutput=True)
    out_o = nc.declare_dram_parameter("out_o", [SQ, OUT], F32, isOutput=True)

    vspill = nc.dram_tensor("vspill", [NH, S, HID], BF16)
    ascr = nc.dram_tensor("ascr", [NH, SQ, S], BF16)

    NQT = SQ // P           # 4 query tiles
    NKT = S // P            # 16 key tiles
    KI = INP // P           # 8 contraction chunks

    with tile.TileContext(nc) as tc:
        from contextlib import ExitStack

        with ExitStack() as stk:
            constp = stk.enter_context(tc.tile_pool(name="const", bufs=1))
            ktp = stk.enter_context(tc.tile_pool(name="kt", bufs=1))

            # ---- constants ----
            id32_t = constp.tile([P, P], F32)
            nc.sync.dma_start(id32_t[:], ident32[:, :])
            idbf_t = constp.tile([P, P], BF16)
            nc.sync.dma_start(idbf_t[:], identbf[:, :])
            vbias_t = constp.tile([P, NH * HID], F32)
            nc.sync.dma_start(vbias_t[:], vbias[:, :])
            obias_t = constp.tile([P, OUT], F32)
            nc.sync.dma_start(obias_t[:], obias[:, :])
            # packed per-partition vectors: column ti/qt holds that tile's vec
            qscale_t = constp.tile([P, SQ // P], F32)
            nc.sync.dma_start(qscale_t[:], qscale[:, :])
            bq_t = constp.tile([P, INP // P], F32)
            nc.sync.dma_start(bq_t[:], bq[:, :])
            bk_t = constp.tile([P, INP // P], F32)
            nc.sync.dma_start(bk_t[:], bk[:, :])

            # ---- phase 1: xT load + V projection (spilled to DRAM) ----
            with ExitStack() as ph1:
                xtp = ph1.enter_context(tc.tile_pool(name="xt", bufs=1))
                xt_t = []
                for ic in range(KI):
                    t = xtp.tile([P, S], BF16, tag=f"xt{ic}")
                    nc.sync.dma_start(t[:], xT[ic * P:(ic + 1) * P, :])
                    xt_t.append(t)

                with ExitStack() as phv:
                    wvp = phv.enter_context(tc.tile_pool(name="wv", bufs=1))
                    vps = phv.enter_context(
                        tc.tile_pool(name="vps", bufs=2, space="PSUM"))
                    vst = phv.enter_context(tc.tile_pool(name="vst", bufs=3))
                    wv_t = []
                    for ic in range(KI):
                        t = wvp.tile([P, NH * HID], BF16, tag=f"wv{ic}")
                        nc.sync.dma_start(t[:], wv[ic * P:(ic + 1) * P, :])
                        wv_t.append(t)
                    for kc in range(NKT):
                        ps = vps.tile([P, NH * HID], F32, tag="vps")
                        for c in range(2):
                            cs = slice(c * 512, (c + 1) * 512)
                            for ic in range(KI):
                                nc.tensor.matmul(
                                    ps[:, cs],
                                    lhsT=xt_t[ic][:, kc * P:(kc + 1) * P],
                                    rhs=wv_t[ic][:, cs],
                                    start=(ic == 0), stop=(ic == KI - 1))
                        vsb = vst.tile([P, NH * HID], BF16, tag="vsb")
                        nc.vector.tensor_add(vsb[:], ps[:], vbias_t[:])
                        nc.sync.dma_start(
                            vspill[:, kc * P:(kc + 1) * P, :]
                            .rearrange("h k d -> k h d"),
                            vsb[:].rearrange("p (h d) -> p h d", d=HID))

                # ---- phase 2: K^T projection (keep in SBUF) ----
                tc.strict_bb_all_engine_barrier()
                with ExitStack() as phk:
                    wkp = phk.enter_context(tc.tile_pool(name="wk", bufs=1))
                    kps = phk.enter_context(
                        tc.tile_pool(name="kps", bufs=2, space="PSUM"))
                    wk_t = []
                    for ic in range(KI):
                        t = wkp.tile([P, NH * HID], BF16, tag=f"wk{ic}")
                        nc.sync.dma_start(t[:], wk[ic * P:(ic + 1) * P, :])
                        wk_t.append(t)
                    for ti in range(KI):
                        kt_tile = ktp.tile([P, S], BF16, tag=f"kt{ti}")
                        for half in range(2):
                            ps = kps.tile([P, 1024], F32, tag="kps")
                            for c in range(2):
                                cs = slice(half * 1024 + c * 512,
                                           half * 1024 + (c + 1) * 512)
                                pcs = slice(c * 512, (c + 1) * 512)
                                for ic in range(KI):
                                    nc.tensor.matmul(
                                        ps[:, pcs],
                                        lhsT=wk_t[ic][:, ti * P:(ti + 1) * P],
                                        rhs=xt_t[ic][:, cs],
                                        start=(ic == 0), stop=(ic == KI - 1))
                            nc.vector.tensor_scalar_add(
                                kt_tile[:, half * 1024:(half + 1) * 1024],
                                ps[:], bk_t[:, ti:ti + 1])
                        if ti == 0:
                            kt_t = []
                        kt_t.append(kt_tile)

            # ---- phase 3: Q^T projection (xq input; xT freed) ----
            tc.strict_bb_all_engine_barrier()
            qtp = stk.enter_context(tc.tile_pool(name="qt", bufs=1))
            with ExitStack() as phq:
                xqp = phq.enter_context(tc.tile_pool(name="xq", bufs=1))
                wqp = phq.enter_context(tc.tile_pool(name="wq", bufs=1))
                qps = phq.enter_context(
                    tc.tile_pool(name="qps", bufs=2, space="PSUM"))
                xq_t = []
                for ic in range(KI):
                    t = xqp.tile([P, SQ], BF16, tag=f"xq{ic}")
                    nc.sync.dma_start(t[:], xq[ic * P:(ic + 1) * P, :])
                    xq_t.append(t)
                wq_t = []
                for ic in range(KI):
                    t = wqp.tile([P, NH * HID], BF16, tag=f"wq{ic}")
                    nc.sync.dma_start(t[:], wq[ic * P:(ic + 1) * P, :])
                    wq_t.append(t)
                qt_t = []
                for ti in range(KI):
                    q_tile = qtp.tile([P, SQ], BF16, tag=f"qt{ti}")
                    ps = qps.tile([P, SQ], F32, tag="qps")
                    for ic in range(KI):
                        nc.tensor.matmul(
                            ps[:],
                            lhsT=wq_t[ic][:, ti * P:(ti + 1) * P],
                            rhs=xq_t[ic][:],
                            start=(ic == 0), stop=(ic == KI - 1))
                    nc.vector.tensor_scalar_add(q_tile[:], ps[:], bq_t[:, ti:ti + 1])
                    qt_t.append(q_tile)

            # ---- phase 4: attention per head ----
            tc.strict_bb_all_engine_barrier()
            o2p = stk.enter_context(tc.tile_pool(name="o2", bufs=1))
            with ExitStack() as pha:
                ep = pha.enter_context(tc.tile_pool(name="e", bufs=6))
                rows = pha.enter_context(tc.tile_pool(name="rows", bufs=2))
                sps = pha.enter_context(
                    tc.tile_pool(name="sps", bufs=2, space="PSUM"))
                o2ps = pha.enter_context(
                    tc.tile_pool(name="o2ps", bufs=2, space="PSUM"))
                atsb = pha.enter_context(tc.tile_pool(name="atsb", bufs=6))
                vld = pha.enter_context(tc.tile_pool(name="vld", bufs=3))
                maskp = pha.enter_context(tc.tile_pool(name="maskp", bufs=1))
                abfp = pha.enter_context(tc.tile_pool(name="abf", bufs=4))

                mask_t = []
                for qt in range(NQT):
                    m = maskp.tile([P, S], BF16, tag=f"mask{qt}",
                                   name=f"mask{qt}")
                    nc.sync.dma_start(m[:], maskM[qt * P:(qt + 1) * P, :])
                    mask_t.append(m)

                o2sb_t = [o2p.tile([P, SQ], BF16, tag=f"o2sb{ti}",
                                   name=f"o2sb{ti}")
                          for ti in range(KI)]

                rw_tiles = {}

                def s_chunk(h, qt):
                    """scores+exp+norm+attn-DMA+bf16-cast for one q-tile."""
                    ti, hh = divmod(h, 2)
                    drow = slice(64 * hh, 64 * hh + 64)
                    if qt == 0:
                        rw_tiles[h] = rows.tile([P, 16], F32, tag="rows",
                                                name=f"rw{h}")
                    rw = rw_tiles[h]
                    e = ep.tile([P, S], F32, tag="e", name=f"e{h}_{qt}")
                    for half in range(2):
                        ps = sps.tile([P, 1024], F32, tag="sps",
                                      name=f"sps{h}_{qt}_{half}")
                        hcs = slice(half * 1024, (half + 1) * 1024)
                        for c in range(2):
                            cs = slice(half * 1024 + c * 512,
                                       half * 1024 + (c + 1) * 512)
                            pcs = slice(c * 512, (c + 1) * 512)
                            nc.tensor.matmul(
                                ps[:, pcs],
                                lhsT=qt_t[ti][drow, qt * P:(qt + 1) * P],
                                rhs=kt_t[ti][drow, cs],
                                start=True, stop=True)
                        nc.vector.tensor_add(ps[:], ps[:], mask_t[qt][:, hcs])
                        nc.scalar.activation(
                            e[:, half * 1024:(half + 1) * 1024], ps[:],
                            mybir.ActivationFunctionType.Exp,
                            scale=qscale_t[:, qt:qt + 1],
                            accum_out=rw[:, qt * 4 + half:qt * 4 + half + 1])
                    nc.vector.tensor_add(
                        rw[:, qt * 4 + 2:qt * 4 + 3],
                        rw[:, qt * 4 + 0:qt * 4 + 1],
                        rw[:, qt * 4 + 1:qt * 4 + 2])
                    nc.vector.reciprocal(
                        rw[:, qt * 4 + 3:qt * 4 + 4],
                        rw[:, qt * 4 + 2:qt * 4 + 3])
                    nc.vector.tensor_scalar_mul(
                        e[:], e[:], rw[:, qt * 4 + 3:qt * 4 + 4])
                    nc.sync.dma_start(attn_o[h, qt * P:(qt + 1) * P, :], e[:])
                    abf = abfp.tile([P, S], BF16, tag="abf",
                                    name=f"abf{h}_{qt}")
                    nc.vector.tensor_copy(abf[:], e[:])
                    nc.sync.dma_start(ascr[h, qt * P:(qt + 1) * P, :], abf[:])
                    return abf

                # software pipeline: head h's transpose/AV loop carries head
                # h+1's score chunks in program order so the PE stream mixes
                # regular matmuls into the transpose phase.
                abf_h = [s_chunk(0, qt) for qt in range(NQT)]
                for h in range(NH):
                    ti, hh = divmod(h, 2)
                    drow = slice(64 * hh, 64 * hh + 64)
                    abf_next = []
                    o2 = o2ps.tile([64, 512], F32, tag="o2", name=f"o2_{h}")
                    for kc in range(NKT):
                        vt = vld.tile([P, HID], BF16, tag="vt",
                                      name=f"vt{h}_{kc}")
                        nc.sync.dma_start(
                            vt[:], vspill[h, kc * P:(kc + 1) * P, :])
                        asb = atsb.tile([P, SQ], BF16, tag="asb",
                                        name=f"asb{h}_{kc}")
                        nc.sync.dma_start(
                            asb[:], ascr[h, :, kc * P:(kc + 1) * P],
                            transpose=True)
                        nc.tensor.matmul(
                            o2[:], lhsT=vt[:],
                            rhs=asb[:],
                            start=(kc == 0), stop=(kc == NKT - 1))
                        if h + 1 < NH and kc % 4 == 3:
                            abf_next.append(s_chunk(h + 1, kc // 4))
                    nc.any.tensor_copy(o2sb_t[ti][drow, :], o2[:])
                    abf_h = abf_next

            # ---- phase 5: output projection ----
            tc.strict_bb_all_engine_barrier()
            with ExitStack() as pho:
                wop = pho.enter_context(tc.tile_pool(name="wo", bufs=1))
                ops = pho.enter_context(
                    tc.tile_pool(name="ops", bufs=2, space="PSUM"))
                osb = pho.enter_context(tc.tile_pool(name="osb", bufs=2))
                wo_t = []
                for ti in range(KI):
                    t = wop.tile([P, OUT], BF16, tag=f"wo{ti}")
                    nc.sync.dma_start(t[:], wo[ti * P:(ti + 1) * P, :])
                    wo_t.append(t)
                for qt in range(NQT):
                    ps = ops.tile([P, OUT], F32, tag="ops")
                    for c in range(2):
                        cs = slice(c * 512, (c + 1) * 512)
                        for ti in range(KI):
                            nc.tensor.matmul(
                                ps[:, cs],
                                lhsT=o2sb_t[ti][:, qt * P:(qt + 1) * P],
                                rhs=wo_t[ti][:, cs],
                                start=(ti == 0), stop=(ti == KI - 1))
                    ob = osb.tile([P, OUT], F32, tag="ob")
                    nc.vector.tensor_add(ob[:], ps[:], obias_t[:])
                    nc.sync.dma_start(out_o[qt * P:(qt + 1) * P, :], ob[:])

    if legalize:
        _split_multi_waits(nc)
    return nc


_NC_CACHE = None


def _get_nc():
    global _NC_CACHE
    if _NC_CACHE is None:
        _NC_CACHE = build_nc()
    return _NC_CACHE


def make_in_maps(x, pad_mask, attn_mask, W_Q, b_Q, W_K, b_K, W_V, b_V, W_O, b_O):
    x = np.asarray(x, dtype=np.float32)
    pad_mask = np.asarray(pad_mask)
    attn_mask = np.asarray(attn_mask)
    wq = np.ascontiguousarray(
        np.asarray(W_Q, np.float32)[:, PERM]).astype(ml_dtypes.bfloat16)
    wk = np.ascontiguousarray(
        np.asarray(W_K, np.float32)[:, PERM]).astype(ml_dtypes.bfloat16)
    wv = np.ascontiguousarray(
        np.asarray(W_V, np.float32)[:, PERM]).astype(ml_dtypes.bfloat16)
    wo = np.ascontiguousarray(
        np.asarray(W_O, np.float32)).astype(ml_dtypes.bfloat16)
    bqp = np.ascontiguousarray(np.asarray(b_Q, np.float32)[PERM].reshape(8, P).T)
    bkp = np.ascontiguousarray(np.asarray(b_K, np.float32)[PERM].reshape(8, P).T)
    vb = np.broadcast_to(np.asarray(b_V, np.float32)[PERM][None, :],
                         (P, NH * HID)).copy()
    ob = np.broadcast_to(np.asarray(b_O, np.float32)[None, :], (P, OUT)).copy()
    id32 = np.eye(P, dtype=np.float32)
    idbf = np.eye(P, dtype=ml_dtypes.bfloat16)

    amask = attn_mask[0, 0]  # [S, S] bool
    in_maps = []
    for c in range(NCORES):
        b, qb = divmod(c, 4)
        q0 = qb * SQ
        xTb = np.ascontiguousarray(x[b].T).astype(ml_dtypes.bfloat16)
        mm = (amask[q0:q0 + SQ, :].astype(np.float32) * NEG).astype(
            ml_dtypes.bfloat16)
        qs = np.ascontiguousarray(
            (0.125 * (1.0 - pad_mask[b, q0:q0 + SQ].astype(np.float32)))
            .astype(np.float32).reshape(4, P).T)
        in_maps.append({
            "xT": xTb,
            "xq": np.ascontiguousarray(xTb[:, q0:q0 + SQ]),
            "wq": wq, "wk": wk, "wv": wv, "wo": wo,
            "bq": bqp, "bk": bkp, "vbias": vb, "obias": ob,
            "maskM": mm, "qscale": qs,
            "ident32": id32, "identbf": idbf,
        })
    return in_maps


def kernel(x, pad_mask, attn_mask, W_Q, b_Q, W_K, b_K, W_V, b_V, W_O, b_O):
    nc = _get_nc()
    in_maps = make_in_maps(x, pad_mask, attn_mask, W_Q, b_Q, W_K, b_K,
                           W_V, b_V, W_O, b_O)
    res = run_bass_kernel_spmd(nc, in_maps, list(range(NCORES)))
    attn = np.empty((B, NH, S, S), dtype=np.float32)
    out = np.empty((B, S, OUT), dtype=np.float32)
    for c in range(NCORES):
        b, qb = divmod(c, 4)
        q0 = qb * SQ
        attn[b, :, q0:q0 + SQ, :] = res.results[c]["attn_o"]
        out[b, q0:q0 + SQ, :] = res.results[c]["out_o"]
    return attn, out


# revision 28
# speedup vs baseline: 2.1217x; 2.1217x over previous
"""Trainium2 Bass kernel for multi-head self-attention (B=2, S=2048, 16 heads).

Sharding: 8 cores = (2 batches) x (4 query-blocks of 512). Each core computes
Q for its 512 queries against all 2048 keys for all 16 heads, plus the output
projection rows for its queries. Outputs are disjoint shards -> no collectives.

Per-core layout notes:
  - x is fed pre-transposed (xT [1024, 2048]) so projections need no on-chip
    transpose of the activations.
  - W_Q/W_K/W_V columns (and b_*) are permuted on host to head-major order
    (reference uses head-minor "(h n)" packing).
  - softmax runs without row-max subtraction (scores are bounded; masked
    entries are exactly -1e26 -> exp -> 0). Pad-masked query rows use a
    per-partition exp scale of 0 so every entry becomes exp(0)=1 and the row
    normalizes to exactly 1/2048, matching the reference bit pattern.
"""

import os
import sys

sys.path.insert(0, "/opt/trn_rl_repo")

import numpy as np
import ml_dtypes

import concourse.bass as bass
import concourse.mybir as mybir
import concourse.tile as tile
from concourse.bass_utils import run_bass_kernel_spmd

B, S, INP, HID, NH, OUT = 2, 2048, 1024, 64, 16, 1024
NEG = np.float32(-1e26)
SQ = 512          # queries per core
NCORES = 8
P = 128

F32 = mybir.dt.float32
F32R = mybir.dt.float32r
BF16 = mybir.dt.bfloat16

# head-major permutation: permuted col h*64+d  <- original col d*16+h
PERM = np.array([d * NH + h for h in range(NH) for d in range(HID)], dtype=np.int64)


def _split_multi_waits(nc):
    """Legalize sync waits for this walrus build: every instruction except
    Drain may carry at most 1 sem wait (EventSemaphore: 2). Excess waits are
    hoisted onto same-engine NOPs inserted immediately before the instruction
    (engine blocks on each in turn -> identical semantics)."""
    import bass_rust

    cnt = [0]
    for f in nc.m.functions:
        for blk in f.blocks:
            il = blk.instructions
            i = 0
            while i < len(il):
                ins = il[i]
                tn = type(ins).__name__
                si = ins.sync_info
                cap = 2 if tn == "InstEventSemaphore" else 1
                if si is None or len(si.on_wait) <= cap:
                    i += 1
                    continue
                waits = list(si.on_wait)
                extra, kept = waits[:-cap], waits[-cap:]
                ins.sync_info = bass_rust.SyncInfo(
                    on_wait=kept, on_update=si.on_update)
                for w in extra:
                    cnt[0] += 1
                    nop = mybir.InstNoOp(
                        name=f"I-waitsplit-{cnt[0]}",
                        engine=ins.engine,
                        ins=[], outs=[],
                        sync_info=bass_rust.SyncInfo(on_wait=[w], on_update=[]))
                    il.insert(i, nop)
                    i += 1
                i += 1
    return cnt[0]


def build_nc(legalize=True):
    nc = bass.Bass()

    xT = nc.declare_dram_parameter("xT", [INP, S], BF16, isOutput=False)
    xq = nc.declare_dram_parameter("xq", [INP, SQ], BF16, isOutput=False)
    wq = nc.declare_dram_parameter("wq", [INP, NH * HID], BF16, isOutput=False)
    wk = nc.declare_dram_parameter("wk", [INP, NH * HID], BF16, isOutput=False)
    wv = nc.declare_dram_parameter("wv", [INP, NH * HID], BF16, isOutput=False)
    wo = nc.declare_dram_parameter("wo", [NH * HID, OUT], BF16, isOutput=False)
    bq = nc.declare_dram_parameter("bq", [P, INP // P], F32, isOutput=False)
    bk = nc.declare_dram_parameter("bk", [P, INP // P], F32, isOutput=False)
    vbias = nc.declare_dram_parameter("vbias", [P, NH * HID], F32, isOutput=False)
    obias = nc.declare_dram_parameter("obias", [P, OUT], F32, isOutput=False)
    maskM = nc.declare_dram_parameter("maskM", [SQ, S], BF16, isOutput=False)
    qscale = nc.declare_dram_parameter("qscale", [P, SQ // P], F32, isOutput=False)
    ident32 = nc.declare_dram_parameter("ident32", [P, P], F32, isOutput=False)
    identbf = nc.declare_dram_parameter("identbf", [P, P], BF16, isOutput=False)

    attn_o = nc.declare_dram_parameter("attn_o", [NH, SQ, S], F32, isOutput=True)
    out_o = nc.declare_dram_parameter("out_o", [SQ, OUT], F32, isOutput=True)

    vspill = nc.dram_tensor("vspill", [NH, S, HID], BF16)

    NQT = SQ // P           # 4 query tiles
    NKT = S // P            # 16 key tiles
    KI = INP // P           # 8 contraction chunks

    with tile.TileContext(nc) as tc:
        from contextlib import ExitStack

        with ExitStack() as stk:
            constp = stk.enter_context(tc.tile_pool(name="const", bufs=1))
            ktp = stk.enter_context(tc.tile_pool(name="kt", bufs=1))

            # ---- constants ----
            id32_t = constp.tile([P, P], F32)
            nc.sync.dma_start(id32_t[:], ident32[:, :])
            idbf_t = constp.tile([P, P], BF16)
            nc.sync.dma_start(idbf_t[:], identbf[:, :])
            vbias_t = constp.tile([P, NH * HID], F32)
            nc.sync.dma_start(vbias_t[:], vbias[:, :])
            obias_t = constp.tile([P, OUT], F32)
            nc.sync.dma_start(obias_t[:], obias[:, :])
            # packed per-partition vectors: column ti/qt holds that tile's vec
            qscale_t = constp.tile([P, SQ // P], F32)
            nc.sync.dma_start(qscale_t[:], qscale[:, :])
            bq_t = constp.tile([P, INP // P], F32)
            nc.sync.dma_start(bq_t[:], bq[:, :])
            bk_t = constp.tile([P, INP // P], F32)
            nc.sync.dma_start(bk_t[:], bk[:, :])

            # ---- phase 1: xT load + V projection (spilled to DRAM) ----
            with ExitStack() as ph1:
                xtp = ph1.enter_context(tc.tile_pool(name="xt", bufs=1))
                xt_t = []
                for ic in range(KI):
                    t = xtp.tile([P, S], BF16, tag=f"xt{ic}")
                    nc.sync.dma_start(t[:], xT[ic * P:(ic + 1) * P, :])
                    xt_t.append(t)

                with ExitStack() as phv:
                    wvp = phv.enter_context(tc.tile_pool(name="wv", bufs=1))
                    vps = phv.enter_context(
                        tc.tile_pool(name="vps", bufs=2, space="PSUM"))
                    vst = phv.enter_context(tc.tile_pool(name="vst", bufs=3))
                    wv_t = []
                    for ic in range(KI):
                        t = wvp.tile([P, NH * HID], BF16, tag=f"wv{ic}")
                        nc.sync.dma_start(t[:], wv[ic * P:(ic + 1) * P, :])
                        wv_t.append(t)
                    for kc in range(NKT):
                        ps = vps.tile([P, NH * HID], F32, tag="vps")
                        for c in range(2):
                            cs = slice(c * 512, (c + 1) * 512)
                            for ic in range(KI):
                                nc.tensor.matmul(
                                    ps[:, cs],
                                    lhsT=xt_t[ic][:, kc * P:(kc + 1) * P],
                                    rhs=wv_t[ic][:, cs],
                                    start=(ic == 0), stop=(ic == KI - 1))
                        vsb = vst.tile([P, NH * HID], BF16, tag="vsb")
                        nc.vector.tensor_add(vsb[:], ps[:], vbias_t[:])
                        nc.sync.dma_start(
                            vspill[:, kc * P:(kc + 1) * P, :]
                            .rearrange("h k d -> k h d"),
                            vsb[:].rearrange("p (h d) -> p h d", d=HID))

                # ---- phase 2: K^T projection (keep in SBUF) ----
                tc.strict_bb_all_engine_barrier()
                with ExitStack() as phk:
                    wkp = phk.enter_context(tc.tile_pool(name="wk", bufs=1))
                    kps = phk.enter_context(
                        tc.tile_pool(name="kps", bufs=2, space="PSUM"))
                    wk_t = []
                    for ic in range(KI):
                        t = wkp.tile([P, NH * HID], BF16, tag=f"wk{ic}")
                        nc.sync.dma_start(t[:], wk[ic * P:(ic + 1) * P, :])
                        wk_t.append(t)
                    for ti in range(KI):
                        kt_tile = ktp.tile([P, S], BF16, tag=f"kt{ti}")
                        for half in range(2):
                            ps = kps.tile([P, 1024], F32, tag="kps")
                            for c in range(2):
                                cs = slice(half * 1024 + c * 512,
                                           half * 1024 + (c + 1) * 512)
                                pcs = slice(c * 512, (c + 1) * 512)
                                for ic in range(KI):
                                    nc.tensor.matmul(
                                        ps[:, pcs],
                                        lhsT=wk_t[ic][:, ti * P:(ti + 1) * P],
                                        rhs=xt_t[ic][:, cs],
                                        start=(ic == 0), stop=(ic == KI - 1))
                            nc.vector.tensor_scalar_add(
                                kt_tile[:, half * 1024:(half + 1) * 1024],
                                ps[:], bk_t[:, ti:ti + 1])
                        if ti == 0:
                            kt_t = []
                        kt_t.append(kt_tile)

            # ---- phase 3: Q^T projection (xq input; xT freed) ----
            tc.strict_bb_all_engine_barrier()
            qtp = stk.enter_context(tc.tile_pool(name="qt", bufs=1))
            with ExitStack() as phq:
                xqp = phq.enter_context(tc.tile_pool(name="xq", bufs=1))
                wqp = phq.enter_context(tc.tile_pool(name="wq", bufs=1))
                qps = phq.enter_context(
                    tc.tile_pool(name="qps", bufs=2, space="PSUM"))
                xq_t = []
                for ic in range(KI):
                    t = xqp.tile([P, SQ], BF16, tag=f"xq{ic}")
                    nc.sync.dma_start(t[:], xq[ic * P:(ic + 1) * P, :])
                    xq_t.append(t)
                wq_t = []
                for ic in range(KI):
                    t = wqp.tile([P, NH * HID], BF16, tag=f"wq{ic}")
                    nc.sync.dma_start(t[:], wq[ic * P:(ic + 1) * P, :])
                    wq_t.append(t)
                qt_t = []
                for ti in range(KI):
                    q_tile = qtp.tile([P, SQ], BF16, tag=f"qt{ti}")
                    ps = qps.tile([P, SQ], F32, tag="qps")
                    for ic in range(KI):
                        nc.tensor.matmul(
                            ps[:],
                            lhsT=wq_t[ic][:, ti * P:(ti + 1) * P],
                            rhs=xq_t[ic][:],
                            start=(ic == 0), stop=(ic == KI - 1))
                    nc.vector.tensor_scalar_add(q_tile[:], ps[:], bq_t[:, ti:ti + 1])
                    qt_t.append(q_tile)

            # ---- phase 4: attention per head ----
            tc.strict_bb_all_engine_barrier()
            o2p = stk.enter_context(tc.tile_pool(name="o2", bufs=1))
            with ExitStack() as pha:
                ep = pha.enter_context(tc.tile_pool(name="e", bufs=6))
                rows = pha.enter_context(tc.tile_pool(name="rows", bufs=2))
                sps = pha.enter_context(
                    tc.tile_pool(name="sps", bufs=2, space="PSUM"))
                atps = pha.enter_context(
                    tc.tile_pool(name="atps", bufs=2, space="PSUM"))
                o2ps = pha.enter_context(
                    tc.tile_pool(name="o2ps", bufs=2, space="PSUM"))
                atsb = pha.enter_context(tc.tile_pool(name="atsb", bufs=6))
                vld = pha.enter_context(tc.tile_pool(name="vld", bufs=3))
                maskp = pha.enter_context(tc.tile_pool(name="maskp", bufs=1))
                abfp = pha.enter_context(tc.tile_pool(name="abf", bufs=8))

                mask_t = []
                for qt in range(NQT):
                    m = maskp.tile([P, S], BF16, tag=f"mask{qt}",
                                   name=f"mask{qt}")
                    nc.sync.dma_start(m[:], maskM[qt * P:(qt + 1) * P, :])
                    mask_t.append(m)

                o2sb_t = [o2p.tile([P, SQ], BF16, tag=f"o2sb{ti}",
                                   name=f"o2sb{ti}")
                          for ti in range(KI)]

                rw_tiles = {}

                def s_chunk(h, qt):
                    """scores+exp+norm+attn-DMA+bf16-cast for one q-tile."""
                    ti, hh = divmod(h, 2)
                    drow = slice(64 * hh, 64 * hh + 64)
                    if qt == 0:
                        rw_tiles[h] = rows.tile([P, 16], F32, tag="rows",
                                                name=f"rw{h}")
                    rw = rw_tiles[h]
                    e = ep.tile([P, S], F32, tag="e", name=f"e{h}_{qt}")
                    for half in range(2):
                        ps = sps.tile([P, 1024], F32, tag="sps",
                                      name=f"sps{h}_{qt}_{half}")
                        hcs = slice(half * 1024, (half + 1) * 1024)
                        for c in range(2):
                            cs = slice(half * 1024 + c * 512,
                                       half * 1024 + (c + 1) * 512)
                            pcs = slice(c * 512, (c + 1) * 512)
                            nc.tensor.matmul(
                                ps[:, pcs],
                                lhsT=qt_t[ti][drow, qt * P:(qt + 1) * P],
                                rhs=kt_t[ti][drow, cs],
                                start=True, stop=True)
                        nc.vector.tensor_add(ps[:], ps[:], mask_t[qt][:, hcs])
                        nc.scalar.activation(
                            e[:, half * 1024:(half + 1) * 1024], ps[:],
                            mybir.ActivationFunctionType.Exp,
                            scale=qscale_t[:, qt:qt + 1],
                            accum_out=rw[:, qt * 4 + half:qt * 4 + half + 1])
                    nc.vector.tensor_add(
                        rw[:, qt * 4 + 2:qt * 4 + 3],
                        rw[:, qt * 4 + 0:qt * 4 + 1],
                        rw[:, qt * 4 + 1:qt * 4 + 2])
                    nc.vector.reciprocal(
                        rw[:, qt * 4 + 3:qt * 4 + 4],
                        rw[:, qt * 4 + 2:qt * 4 + 3])
                    nc.vector.tensor_scalar_mul(
                        e[:], e[:], rw[:, qt * 4 + 3:qt * 4 + 4])
                    nc.sync.dma_start(attn_o[h, qt * P:(qt + 1) * P, :], e[:])
                    abf = abfp.tile([P, S], BF16, tag="abf",
                                    name=f"abf{h}_{qt}")
                    nc.vector.tensor_copy(abf[:], e[:])
                    return abf

                # software pipeline: head h's transpose/AV loop carries head
                # h+1's score chunks in program order so the PE stream mixes
                # regular matmuls into the transpose phase.
                abf_h = [s_chunk(0, qt) for qt in range(NQT)]
                for h in range(NH):
                    ti, hh = divmod(h, 2)
                    drow = slice(64 * hh, 64 * hh + 64)
                    abf_next = []
                    o2 = o2ps.tile([64, 512], F32, tag="o2", name=f"o2_{h}")
                    for kc in range(NKT):
                        vt = vld.tile([P, HID], BF16, tag="vt",
                                      name=f"vt{h}_{kc}")
                        nc.sync.dma_start(
                            vt[:], vspill[h, kc * P:(kc + 1) * P, :])
                        aps = atps.tile([P, SQ], F32, tag="aps",
                                        name=f"aps{h}_{kc}")
                        for qt in range(NQT):
                            nc.tensor.matmul(
                                aps[:, qt * P:(qt + 1) * P],
                                lhsT=abf_h[qt][:, kc * P:(kc + 1) * P],
                                rhs=idbf_t[:],
                                start=True, stop=True)
                        asb = atsb.tile([P, SQ], BF16, tag="asb",
                                        name=f"asb{h}_{kc}")
                        nc.any.tensor_copy(asb[:], aps[:])
                        nc.tensor.matmul(
                            o2[:], lhsT=vt[:],
                            rhs=asb[:],
                            start=(kc == 0), stop=(kc == NKT - 1))
                        if h + 1 < NH and kc % 4 == 3:
                            abf_next.append(s_chunk(h + 1, kc // 4))
                    nc.any.tensor_copy(o2sb_t[ti][drow, :], o2[:])
                    abf_h = abf_next

            # ---- phase 5: output projection ----
            tc.strict_bb_all_engine_barrier()
            with ExitStack() as pho:
                wop = pho.enter_context(tc.tile_pool(name="wo", bufs=1))
                ops = pho.enter_context(
                    tc.tile_pool(name="ops", bufs=2, space="PSUM"))
                osb = pho.enter_context(tc.tile_pool(name="osb", bufs=2))
                wo_t = []
                for ti in range(KI):
                    t = wop.tile([P, OUT], BF16, tag=f"wo{ti}")
                    nc.sync.dma_start(t[:], wo[ti * P:(ti + 1) * P, :])
                    wo_t.append(t)
                for qt in range(NQT):
                    ps = ops.tile([P, OUT], F32, tag="ops")
                    for c in range(2):
                        cs = slice(c * 512, (c + 1) * 512)
                        for ti in range(KI):
                            nc.tensor.matmul(
                                ps[:, cs],
                                lhsT=o2sb_t[ti][:, qt * P:(qt + 1) * P],
                                rhs=wo_t[ti][:, cs],
                                start=(ti == 0), stop=(ti == KI - 1))
                    ob = osb.tile([P, OUT], F32, tag="ob")
                    nc.vector.tensor_add(ob[:], ps[:], obias_t[:])
                    nc.sync.dma_start(out_o[qt * P:(qt + 1) * P, :], ob[:])

    if legalize:
        _split_multi_waits(nc)
    return nc


_NC_CACHE = None


def _get_nc():
    global _NC_CACHE
    if _NC_CACHE is None:
        _NC_CACHE = build_nc()
    return _NC_CACHE


def make_in_maps(x, pad_mask, attn_mask, W_Q, b_Q, W_K, b_K, W_V, b_V, W_O, b_O):
    x = np.asarray(x, dtype=np.float32)
    pad_mask = np.asarray(pad_mask)
    attn_mask = np.asarray(attn_mask)
    wq = np.ascontiguousarray(
        np.asarray(W_Q, np.float32)[:, PERM]).astype(ml_dtypes.bfloat16)
    wk = np.ascontiguousarray(
        np.asarray(W_K, np.float32)[:, PERM]).astype(ml_dtypes.bfloat16)
    wv = np.ascontiguousarray(
        np.asarray(W_V, np.float32)[:, PERM]).astype(ml_dtypes.bfloat16)
    wo = np.ascontiguousarray(
        np.asarray(W_O, np.float32)).astype(ml_dtypes.bfloat16)
    bqp = np.ascontiguousarray(np.asarray(b_Q, np.float32)[PERM].reshape(8, P).T)
    bkp = np.ascontiguousarray(np.asarray(b_K, np.float32)[PERM].reshape(8, P).T)
    vb = np.broadcast_to(np.asarray(b_V, np.float32)[PERM][None, :],
                         (P, NH * HID)).copy()
    ob = np.broadcast_to(np.asarray(b_O, np.float32)[None, :], (P, OUT)).copy()
    id32 = np.eye(P, dtype=np.float32)
    idbf = np.eye(P, dtype=ml_dtypes.bfloat16)

    amask = attn_mask[0, 0]  # [S, S] bool
    in_maps = []
    for c in range(NCORES):
        b, qb = divmod(c, 4)
        q0 = qb * SQ
        xTb = np.ascontiguousarray(x[b].T).astype(ml_dtypes.bfloat16)
        mm = (amask[q0:q0 + SQ, :].astype(np.float32) * NEG).astype(
            ml_dtypes.bfloat16)
        qs = np.ascontiguousarray(
            (0.125 * (1.0 - pad_mask[b, q0:q0 + SQ].astype(np.float32)))
            .astype(np.float32).reshape(4, P).T)
        in_maps.append({
            "xT": xTb,
            "xq": np.ascontiguousarray(xTb[:, q0:q0 + SQ]),
            "wq": wq, "wk": wk, "wv": wv, "wo": wo,
            "bq": bqp, "bk": bkp, "vbias": vb, "obias": ob,
            "maskM": mm, "qscale": qs,
            "ident32": id32, "identbf": idbf,
        })
    return in_maps


def kernel(x, pad_mask, attn_mask, W_Q, b_Q, W_K, b_K, W_V, b_V, W_O, b_O):
    nc = _get_nc()
    in_maps = make_in_maps(x, pad_mask, attn_mask, W_Q, b_Q, W_K, b_K,
                           W_V, b_V, W_O, b_O)
    res = run_bass_kernel_spmd(nc, in_maps, list(range(NCORES)))
    attn = np.empty((B, NH, S, S), dtype=np.float32)
    out = np.empty((B, S, OUT), dtype=np.float32)
    for c in range(NCORES):
        b, qb = divmod(c, 4)
        q0 = qb * SQ
        attn[b, :, q0:q0 + SQ, :] = res.results[c]["attn_o"]
        out[b, q0:q0 + SQ, :] = res.results[c]["out_o"]
    return attn, out
